# revision 1
# baseline (speedup 1.0000x reference)
"""Trainium2 Bass kernel for NeuroFusionGNN (2x SAGEConv + LN + GELU + residual).

Self-contained: takes full inputs, shards across 8 NeuronCores internally,
returns the full [20000, 256] float32 output.

Strategy (hardcoded for N=20000, D=256, E=320000, 8 cores):
- Nodes sharded by dst: core c owns rows [c*2500, (c+1)*2500), processed as
  20 windows of 128 dst nodes (last window re-covers rows 2372..2500 and
  stores only its last 68 rows).
- Host preprocessing sorts edges by dst and emits, per (core, window):
  int16 gather indices (src node ids, padded to 128-multiples with idx 0)
  and bf16 "A" matrices A[e, d] = 1/deg(dst_e) one-hot on the window-local
  dst column (padding rows are all-zero).
- Device: dma_gather pulls neighbor rows (bf16) into [128e, 256] tiles;
  PE computes meanT[feat, dst] = sum_t msg_t[:, half].T @ A_t (PSUM f32);
  root term transposes x rows on PE and matmuls in f32 against W_r.T;
  LayerNorm via bn_stats/bn_aggr + Newton rsqrt on DVE; GELU on ACT;
  residual in f32. Between layers an AllGather replicates bf16 x1 to all
  cores for the layer-2 gathers.
"""
import os
import numpy as np
import ml_dtypes

import concourse.bacc as bacc
import concourse.tile as tile
import concourse.mybir as mybir
from concourse import bass_utils

N = 20000
D = 256
NCORES = 8
SHARD = N // NCORES          # 2500
WPC = 20                     # windows per core
WIN = 128
LAST_STORE = SHARD - (WPC - 1) * WIN   # 68
LAST_BASE = SHARD - WIN                # 2372
LN_EPS = 1e-5
GELU_MODE = "gelu"           # "gelu" (ACT table) or "erf" (erf-based exact)
GCHUNK = int(os.environ.get("GCHUNK", "8"))      # edge-tiles per dma_gather call
NSWQ = int(os.environ.get("NSWQ", "4"))          # SWDGE queues to spread gathers over

f32 = mybir.dt.float32
bf16 = mybir.dt.bfloat16
i16 = mybir.dt.int16
i32 = mybir.dt.int32
Alu = mybir.AluOpType

_cache = {}


def _chunks(T):
    """Per-window gather chunk boundaries (one chunk per SWDGE queue,
    each <= 8 tiles / 1024 idxs)."""
    nq = max(NSWQ, -(-T // GCHUNK))
    b = [round(j * T / nq) for j in range(nq + 1)]
    return [(b[j], b[j + 1]) for j in range(nq) if b[j + 1] > b[j]]


def _preprocess(edge_index):
    src = np.asarray(edge_index[0], dtype=np.int64)
    dst = np.asarray(edge_index[1], dtype=np.int64)
    deg = np.bincount(dst, minlength=N)
    inv = (1.0 / np.maximum(deg, 1)).astype(np.float32)

    order = np.argsort(dst, kind="stable")
    ssrc = src[order]
    sdst = dst[order]

    # store-range boundaries for every (core, window)
    store_lo = np.array([c * SHARD + w * WIN for c in range(NCORES) for w in range(WPC)]
                        + [N], dtype=np.int64)
    starts = np.searchsorted(sdst, store_lo)

    # dedup (src, window): one gathered row per distinct src in a window;
    # its A row carries every dst column (values summed for parallel edges)
    uniq = {}
    ucnt = np.zeros((NCORES, WPC), dtype=np.int64)
    for c in range(NCORES):
        for w in range(WPC):
            k = c * WPC + w
            sl = slice(starts[k], starts[k + 1])
            s_u, r_of = np.unique(ssrc[sl], return_inverse=True)
            uniq[(c, w)] = (s_u, r_of)
            ucnt[c, w] = len(s_u)

    T = np.maximum(1, -(-ucnt.max(axis=0) // 128))         # tiles per window
    offs = np.concatenate([[0], np.cumsum(T)])             # tile offsets
    total_T = int(offs[-1])

    ncalls = sum(len(_chunks(int(t))) for t in T)
    idx_blobs, A_blobs, cnt_blobs = [], [], []
    for c in range(NCORES):
        idxb = np.full((16, total_T * 8), -1, dtype=np.int16)
        Ab = np.zeros((128, total_T * 128), dtype=np.float32)
        cnts = np.zeros(ncalls, dtype=np.int32)
        ci = 0
        for w in range(WPC):
            k = c * WPC + w
            sl = slice(starts[k], starts[k + 1])
            d = sdst[sl]
            s_u, r_of = uniq[(c, w)]
            n = len(s_u)
            base = c * SHARD + (w * WIN if w < WPC - 1 else LAST_BASE)
            col = d - base
            np.add.at(Ab, ((r_of & 127), (offs[w] + (r_of >> 7)) * 128 + col), inv[d])
            r = np.arange(n)
            idxb[r % 16, offs[w] * 8 + r // 16] = s_u.astype(np.int16)
            for (t0, t1) in _chunks(int(T[w])):
                cnt = min(n, t1 * 128) - t0 * 128
                if cnt <= 0:
                    # keep one valid dummy so the call is never empty
                    pos = t0 * 128
                    idxb[pos % 16, offs[w] * 8 + pos // 16] = 0
                    cnt = 1
                cnts[ci] = cnt
                ci += 1
        assert ci == ncalls
        idx_blobs.append(np.tile(idxb, (8, 1)))
        A_blobs.append(Ab.astype(ml_dtypes.bfloat16))
        cnt_blobs.append(np.tile(cnts[None, :], (128, 1)))

    return [int(x) for x in T], [int(x) for x in offs], idx_blobs, A_blobs, cnt_blobs


def _build(Ts, offs, use_b, use_g, use_be, single_core=False, compile=True,
           repeat=1, skip=frozenset(), full_repeat=1):
    """Build + compile the SPMD program. Returns nc.

    single_core=True replaces the AllGather with a local DRAM copy (for
    cost-model timing via TimelineSim, which is single-core only).
    repeat>1 wraps each layer in a device-side For_i loop (timing only)."""
    total_T = sum(Ts)
    nc = bacc.Bacc("TRN2", target_bir_lowering=False, debug=False,
                   num_devices=1 if single_core else NCORES,
                   num_swdge_queues=NSWQ)

    ncalls = sum(len(_chunks(t)) for t in Ts)
    xbf = nc.dram_tensor("xbf", [N, D], bf16, kind="ExternalInput").ap()
    cntd = nc.dram_tensor("cnt", [128, ncalls], i32, kind="ExternalInput").ap()
    xsh = nc.dram_tensor("xsh", [SHARD, D], f32, kind="ExternalInput").ap()
    idxb = nc.dram_tensor("idxb", [128, total_T * 8], i16, kind="ExternalInput").ap()
    Ab = nc.dram_tensor("Ab", [128, total_T * 128], bf16, kind="ExternalInput").ap()
    ident = nc.dram_tensor("ident", [128, 128], f32, kind="ExternalInput").ap()
    # weights: [128, 2, 256]; wl bf16, wr f32 (rhs of the linear matmuls)
    wls = [nc.dram_tensor(f"w{i}l", [128, 2, D], bf16, kind="ExternalInput").ap()
           for i in (1, 2)]
    wrs = [nc.dram_tensor(f"w{i}r", [128, 2, D], f32, kind="ExternalInput").ap()
           for i in (1, 2)]
    bias_in = {}
    for i in (1, 2):
        if use_b:
            bias_in[f"b{i}"] = nc.dram_tensor(f"b{i}", [128, D], f32, kind="ExternalInput").ap()
        if use_g:
            bias_in[f"g{i}"] = nc.dram_tensor(f"g{i}", [128, D], f32, kind="ExternalInput").ap()
        if use_be:
            bias_in[f"be{i}"] = nc.dram_tensor(f"be{i}", [128, D], f32, kind="ExternalInput").ap()
    out = nc.dram_tensor("out", [SHARD, D], f32, kind="ExternalOutput").ap()

    MAGIC = 0x5F3759DF

    with tile.TileContext(nc) as tc:
        with (
            tc.tile_pool(name="cst", bufs=1) as cst,
            tc.tile_pool(name="io", bufs=3) as io,
            tc.tile_pool(name="mid", bufs=3) as mid,
            tc.tile_pool(name="sm", bufs=4) as sm,
            tc.tile_pool(name="ps", bufs=2, space="PSUM") as ps,
            tc.tile_pool(name="dram", bufs=1, space="DRAM") as dram,
        ):
            idx_sb = cst.tile([128, total_T * 8], i16)
            nc.sync.dma_start(idx_sb[:], idxb)
            cnt_sb = cst.tile([128, ncalls], i32)
            nc.sync.dma_start(cnt_sb[:], cntd)
            id_sb = cst.tile([128, 128], f32)
            nc.sync.dma_start(id_sb[:], ident)
            wl_sb, wr_sb, bias_sb = [], [], {}
            for i in (0, 1):
                wl = cst.tile([128, 2, D], bf16, name=f"wl{i}")
                nc.sync.dma_start(wl[:], wls[i])
                wl_sb.append(wl)
                wr = cst.tile([128, 2, D], f32, name=f"wr{i}")
                nc.sync.dma_start(wr[:], wrs[i])
                wr_sb.append(wr)
                for key, use in ((f"b{i+1}", use_b), (f"g{i+1}", use_g), (f"be{i+1}", use_be)):
                    if use:
                        bt = cst.tile([128, D], f32, name=f"bias_{key}")
                        nc.sync.dma_start(bt[:], bias_in[key])
                        bias_sb[key] = bt

            # gather skips trailing-negative-padded rows, so matmuls read
            # stale SBUF there (times all-zero A columns). Zero every msg pool
            # slot once at full extent so stale bytes can never be NaN/Inf.
            for j in range(3):
                mz = io.tile([128, max(Ts), D], bf16, tag="msg", name=f"msgz_{j}")
                nc.vector.memset(mz[:], 0.0)

            x1res = cst.tile([128, WPC, D], f32)
            x1f = dram.tile([WIN, D], f32)   # rows LAST_BASE..SHARD only
            agin = dram.tile([SHARD, D], bf16)
            agout = dram.tile([N, D], bf16)

            def emit_layer(layer, gsrc):
                ci = [0]
                for w in range(WPC):
                    T = Ts[w]
                    off = offs[w]
                    last = w == WPC - 1
                    base = (w * WIN) if not last else LAST_BASE
                    st_lo = w * WIN
                    n_st = WIN if not last else LAST_STORE
                    pofs = 0 if not last else WIN - LAST_STORE

                    msg = io.tile([128, T, D], bf16, tag="msg", name=f"msg_{layer}_{w}")
                    if "gather" in skip:
                        nc.vector.memset(msg[:, 0, 0:8], 0.0)
                    if "gather" not in skip:
                        # Split tiles into NSWQ near-equal chunks (one per SWDGE
                        # queue; Q7 emission parallelizes across queues). Each
                        # chunk stays <= 8 tiles (1024 idxs/call limit). Exact
                        # per-core valid counts come from the cnt blob.
                        for qi, (t0, t1) in enumerate(_chunks(T)):
                            cv = nc.gpsimd.value_load(cnt_sb[0:1, ci[0]:ci[0] + 1])
                            nc.gpsimd.dma_gather(msg[:, t0:t1, :], gsrc,
                                                 idx_sb[:, (off + t0) * 8:(off + t1) * 8],
                                                 (t1 - t0) * 128, cv, D,
                                                 queue_num=qi % NSWQ)
                            ci[0] += 1
                    A_sb = io.tile([128, T * 128], bf16, tag="A", name=f"A_{layer}_{w}")
                    if "aload" in skip:
                        nc.vector.memset(A_sb[:, 0:8], 0.0)
                    if "aload" not in skip:
                        nc.sync.dma_start(A_sb[:], Ab[:, off * 128:(off + T) * 128])
                    if layer == 0:
                        x_sb = io.tile([128, D], f32, tag="x", name=f"x_{layer}_{w}")
                        nc.sync.dma_start(x_sb[:], xsh[base:base + WIN, :])
                        xsl = lambda a, b: x_sb[:, a:b]
                    elif not last:
                        xsl = lambda a, b: x1res[:, w, a:b]
                    else:
                        x_sb = io.tile([128, D], f32, tag="x", name=f"x_{layer}_{w}")
                        nc.sync.dma_start(x_sb[:], x1f[:])
                        xsl = lambda a, b: x_sb[:, a:b]

                    # aggregation: meanT halves accumulate over edge tiles
                    aggps = ps.tile([128, 2, 128], f32, tag="agg", name=f"agg_{layer}_{w}")
                    Teff = 1 if "agg" in skip else T
                    for hh in range(2):
                        for t in range(Teff):
                            nc.tensor.matmul(aggps[:, hh, :],
                                             msg[:, t, hh * 128:(hh + 1) * 128],
                                             A_sb[:, t * 128:(t + 1) * 128],
                                             start=(t == 0), stop=(t == Teff - 1))
                    aggsb = mid.tile([128, 2, 128], bf16, tag="aggsb", name=f"aggsb_{layer}_{w}")
                    nc.scalar.activation(aggsb[:], aggps[:],
                                         mybir.ActivationFunctionType.Copy)

                    # root transpose
                    xtps = ps.tile([128, 2, 128], f32, tag="xt", name=f"xt_{layer}_{w}")
                    for hh in range(2):
                        nc.tensor.transpose(xtps[:, hh, :],
                                            xsl(hh * 128, (hh + 1) * 128), id_sb[:])
                    xtsb = mid.tile([128, 2, 128], f32, tag="xtsb", name=f"xtsb_{layer}_{w}")
                    nc.scalar.activation(xtsb[:], xtps[:],
                                         mybir.ActivationFunctionType.Copy)

                    # linear: h = meanT.T @ WlT + xT.T @ WrT (one mixed-dtype
                    # accumulation group: bf16 agg matmuls + f32 root matmuls)
                    hps = ps.tile([128, D], f32, tag="hP", name=f"hP_{layer}_{w}")
                    nc.tensor.matmul(hps[:], aggsb[:, 0, :], wl_sb[layer][:, 0, :],
                                     start=True, stop=False)
                    nc.tensor.matmul(hps[:], aggsb[:, 1, :], wl_sb[layer][:, 1, :],
                                     start=False, stop=False)
                    nc.tensor.matmul(hps[:], xtsb[:, 0, :], wr_sb[layer][:, 0, :],
                                     start=False, stop=False)
                    nc.tensor.matmul(hps[:], xtsb[:, 1, :], wr_sb[layer][:, 1, :],
                                     start=False, stop=True)

                    if use_b:
                        h_sb = mid.tile([128, D], f32, tag="h", name=f"h_{layer}_{w}")
                        nc.vector.tensor_tensor(h_sb[:], hps[:],
                                                bias_sb[f"b{layer+1}"][:], Alu.add)
                        h_val = h_sb
                    else:
                        h_val = hps

                    # LayerNorm stats
                    st6 = sm.tile([128, 6], f32, tag="st6", name=f"st6_{layer}_{w}")
                    nc.vector.bn_stats(st6[:], h_val[:])
                    mv = sm.tile([128, 2], f32, tag="mv", name=f"mv_{layer}_{w}")
                    nc.vector.bn_aggr(mv[:], st6[:])

                    # inv_std = rsqrt(var + eps): magic + 2 Newton steps (DVE only)
                    va = sm.tile([128, 1], f32, tag="va", name=f"va_{layer}_{w}")
                    nc.vector.tensor_scalar(va[:], mv[:, 1:2], LN_EPS, None, Alu.add)
                    xi = sm.tile([128, 1], i32, tag="xi", name=f"xi_{layer}_{w}")
                    nc.vector.tensor_scalar(xi[:], va[:].bitcast(i32), 1, None,
                                            Alu.arith_shift_right)
                    nc.vector.tensor_scalar(xi[:], xi[:], MAGIC, -1,
                                            Alu.subtract, Alu.mult)
                    rs = sm.tile([128, 1], f32, tag="rs", name=f"rs_{layer}_{w}")
                    nc.vector.tensor_copy(rs[:], xi[:].bitcast(f32))
                    tmp = sm.tile([128, 1], f32, tag="tmp", name=f"tmp_{layer}_{w}")
                    for _ in range(2):
                        nc.vector.tensor_tensor(tmp[:], rs[:], rs[:], Alu.mult)
                        nc.vector.tensor_tensor(tmp[:], tmp[:], va[:], Alu.mult)
                        nc.vector.tensor_scalar(tmp[:], tmp[:], -0.5, 1.5,
                                                Alu.mult, Alu.add)
                        nc.vector.tensor_tensor(rs[:], rs[:], tmp[:], Alu.mult)

                    y = mid.tile([128, D], f32, tag="y", name=f"y_{layer}_{w}")
                    nc.vector.tensor_scalar(y[:], h_val[:], mv[:, 0:1], rs[:],
                                            Alu.subtract, Alu.mult)
                    if use_g:
                        nc.vector.tensor_tensor(y[:], y[:],
                                                bias_sb[f"g{layer+1}"][:], Alu.mult)
                    if use_be:
                        nc.vector.tensor_tensor(y[:], y[:],
                                                bias_sb[f"be{layer+1}"][:], Alu.add)

                    if layer == 0:
                        xn_ap = x1res[:, w, :]
                    else:
                        xn = mid.tile([128, D], f32, tag="xn", name=f"xn_{layer}_{w}")
                        xn_ap = xn[:]
                    if GELU_MODE in ("gelu", "tanh"):
                        fn = (mybir.ActivationFunctionType.Gelu if GELU_MODE == "gelu"
                              else mybir.ActivationFunctionType.Tanh)
                        gl = mid.tile([128, D], f32, tag="gl", name=f"gl_{layer}_{w}")
                        nc.scalar.activation(gl[:], y[:], fn)
                        nc.vector.tensor_tensor(xn_ap, xsl(0, D), gl[:], Alu.add)
                    else:
                        er = mid.tile([128, D], f32, tag="gl", name=f"gl_{layer}_{w}")
                        nc.scalar.activation(er[:], y[:],
                                             mybir.ActivationFunctionType.Erf,
                                             scale=float(1.0 / np.sqrt(2.0)))
                        # z = (er + 1) * y ; xn = 0.5*z + x
                        nc.vector.scalar_tensor_tensor(er[:], er[:], 1.0, y[:],
                                                       Alu.add, Alu.mult)
                        nc.vector.scalar_tensor_tensor(xn_ap, er[:], 0.5, xsl(0, D),
                                                       Alu.mult, Alu.add)

                    if layer == 0:
                        nc.gpsimd.dma_start(agin[st_lo:st_lo + n_st, :],
                                            x1res[pofs:, w, :])
                        if w == WPC - 2:
                            # rows LAST_BASE..(WPC-1)*WIN live in slot w's tail
                            nc.sync.dma_start(
                                x1f[0:WIN - LAST_STORE, :],
                                x1res[WIN - (WIN - LAST_STORE):, w, :])
                        elif last:
                            nc.sync.dma_start(x1f[WIN - LAST_STORE:, :],
                                              x1res[pofs:, w, :])
                    else:
                        nc.sync.dma_start(out[st_lo:st_lo + n_st, :], xn[pofs:, :])

            for rep in range(full_repeat):
                for layer in (0, 1):
                    gsrc = xbf if layer == 0 else agout[:]
                    if repeat > 1:
                        with tc.For_i(0, repeat, 1):
                            emit_layer(layer, gsrc)
                    else:
                        emit_layer(layer, gsrc)

                    if layer == 0:
                        if single_core:
                            nc.sync.dma_start(agout[0:SHARD, :], agin[:])
                        else:
                            nc.gpsimd.collective_compute(
                                "AllGather", Alu.bypass,
                                replica_groups=[list(range(NCORES))],
                                ins=[agin.opt()], outs=[agout.opt()],
                            )

    if compile:
        nc.compile()
    return nc


def _prepare(inputs):
    edge_index = np.asarray(inputs["edge_index"])
    key = hash(edge_index.tobytes())
    if key in _cache:
        return _cache[key]

    Ts, offs, idx_blobs, A_blobs, cnt_blobs = _preprocess(edge_index)

    b1 = np.asarray(inputs["b1l"], dtype=np.float32)
    b2 = np.asarray(inputs["b2l"], dtype=np.float32)
    g1 = np.asarray(inputs["g1"], dtype=np.float32)
    g2 = np.asarray(inputs["g2"], dtype=np.float32)
    be1 = np.asarray(inputs["be1"], dtype=np.float32)
    be2 = np.asarray(inputs["be2"], dtype=np.float32)
    use_b = not (np.all(b1 == 0) and np.all(b2 == 0))
    use_g = not (np.all(g1 == 1) and np.all(g2 == 1))
    use_be = not (np.all(be1 == 0) and np.all(be2 == 0))

    nc = _build(Ts, offs, use_b, use_g, use_be)

    x = np.asarray(inputs["x"], dtype=np.float32)
    xbf = x.astype(ml_dtypes.bfloat16)
    ident = np.eye(128, dtype=np.float32)

    def wdev(W, dtype):
        # rhs[k, j] = W[j, k]; layout [128 part=k%?, 2 khalf, 256 j]
        WT = np.ascontiguousarray(np.asarray(W, dtype=np.float32).T)  # [k, j]
        return WT.reshape(2, 128, D).transpose(1, 0, 2).astype(dtype).copy()

    common = {
        "xbf": xbf,
        "ident": ident,
        "w1l": wdev(inputs["W1l"], ml_dtypes.bfloat16),
        "w2l": wdev(inputs["W2l"], ml_dtypes.bfloat16),
        "w1r": wdev(inputs["W1r"], np.float32),
        "w2r": wdev(inputs["W2r"], np.float32),
    }
    if use_b:
        common["b1"] = np.tile(b1[None, :], (128, 1))
        common["b2"] = np.tile(b2[None, :], (128, 1))
    if use_g:
        common["g1"] = np.tile(g1[None, :], (128, 1))
        common["g2"] = np.tile(g2[None, :], (128, 1))
    if use_be:
        common["be1"] = np.tile(be1[None, :], (128, 1))
        common["be2"] = np.tile(be2[None, :], (128, 1))

    in_maps = []
    for c in range(NCORES):
        m = dict(common)
        m["xsh"] = np.ascontiguousarray(x[c * SHARD:(c + 1) * SHARD, :])
        m["idxb"] = idx_blobs[c]
        m["Ab"] = A_blobs[c]
        m["cnt"] = cnt_blobs[c]
        in_maps.append(m)

    _cache[key] = (nc, in_maps)
    return nc, in_maps


def _assemble(res):
    return np.concatenate([np.asarray(res.results[c]["out"], dtype=np.float32)
                           for c in range(NCORES)], axis=0)


def kernel(**inputs):
    nc, in_maps = _prepare(inputs)
    res = bass_utils.run_bass_kernel_spmd(nc, in_maps, core_ids=list(range(NCORES)))
    return _assemble(res)


def run_traced(**inputs):
    """Returns (output, exec_time_ns or None). For test harness use."""
    nc, in_maps = _prepare(inputs)
    try:
        res = bass_utils.run_bass_kernel_spmd(
            nc, in_maps, core_ids=list(range(NCORES)), trace=True)
        return _assemble(res), res.exec_time_ns
    except Exception as e:  # trace/profile infra can fail independently of the run
        print(f"traced run failed ({e}); falling back to untraced")
        res = bass_utils.run_bass_kernel_spmd(nc, in_maps, core_ids=list(range(NCORES)))
        return _assemble(res), None



# revision 34
# speedup vs baseline: 6.7060x; 6.7060x over previous
"""Trainium2 Bass kernel for NeuroFusionGNN (2x SAGEConv + LN + GELU + residual).

Self-contained: takes full inputs, shards across 8 NeuronCores internally,
returns the full [20000, 256] float32 output.

Strategy (hardcoded for N=20000, D=256, E=320000, 8 cores):
- Nodes sharded by dst: core c owns rows [c*2500, (c+1)*2500), processed as
  20 windows of 128 dst nodes (last window re-covers rows 2372..2500 and
  stores only its last 68 rows).
- Host preprocessing sorts edges by dst and emits, per (core, window):
  int16 gather indices (src node ids, padded to 128-multiples with idx 0)
  and bf16 "A" matrices A[e, d] = 1/deg(dst_e) one-hot on the window-local
  dst column (padding rows are all-zero).
- Device: dma_gather pulls neighbor rows (bf16) into [128e, 256] tiles;
  PE computes meanT[feat, dst] = sum_t msg_t[:, half].T @ A_t (PSUM f32);
  root term transposes x rows on PE and matmuls in f32 against W_r.T;
  LayerNorm via bn_stats/bn_aggr + Newton rsqrt on DVE; GELU on ACT;
  residual in f32. Between layers an AllGather replicates bf16 x1 to all
  cores for the layer-2 gathers.
"""
import os
import numpy as np
import ml_dtypes

import concourse.bacc as bacc
import concourse.tile as tile
import concourse.mybir as mybir
from concourse import bass_utils

N = 20000
D = 256
NCORES = 8
SHARD = N // NCORES          # 2500
WPC = 20                     # windows per core
WIN = 128
LAST_STORE = SHARD - (WPC - 1) * WIN   # 68
LAST_BASE = SHARD - WIN                # 2372
LN_EPS = 1e-5
GELU_MODE = "gelu"           # "gelu" (ACT table) or "erf" (erf-based exact)
GCHUNK = int(os.environ.get("GCHUNK", "8"))      # edge-tiles per dma_gather call
NSWQ = int(os.environ.get("NSWQ", "4"))          # SWDGE queues to spread gathers over
IOBUFS = int(os.environ.get("IOBUFS", "6"))      # io pool depth (gather prefetch)
AGSYNC = int(os.environ.get("AGSYNC", "1"))      # 1: agin store via ACT cast + HWDGE
STATIC_CNT = int(os.environ.get("STATIC_CNT", "1"))  # 1: skip value_load, static count
MSG_FP8 = int(os.environ.get("MSG_FP8", "1"))        # 1: gather node rows in fp8e4m3
SP = int(os.environ.get("SP", "1"))                  # dma_gather single_packet flag
NEWTON = int(os.environ.get("NEWTON", "1"))          # rsqrt Newton iterations
DMASCR = int(os.environ.get("DMASCR", "32768"))      # SWDGE ring carveout bytes
QALT = int(os.environ.get("QALT", "0"))              # 1: 2 big calls/window, queue
                                                     # pairs alternate by window

f32 = mybir.dt.float32
bf16 = mybir.dt.bfloat16
f8 = mybir.dt.float8e4
i16 = mybir.dt.int16
i32 = mybir.dt.int32
Alu = mybir.AluOpType
MSG_DT = f8 if MSG_FP8 else bf16
MSG_NPDT = ml_dtypes.float8_e4m3fn if MSG_FP8 else ml_dtypes.bfloat16

_cache = {}


def _chunks(T):
    """Per-window gather chunk boundaries (one chunk per SWDGE queue,
    each <= 8 tiles / 1024 idxs). With QALT: 2 chunks of <= 8 tiles
    (needs DMASCR >= 32768 for the 2047-descriptor ring capacity)."""
    nq = 2 if QALT else max(NSWQ, -(-T // GCHUNK))
    b = [round(j * T / nq) for j in range(nq + 1)]
    return [(b[j], b[j + 1]) for j in range(nq) if b[j + 1] > b[j]]


def _preprocess(edge_index):
    src = np.asarray(edge_index[0], dtype=np.int64)
    dst = np.asarray(edge_index[1], dtype=np.int64)
    deg = np.bincount(dst, minlength=N)
    inv = (1.0 / np.maximum(deg, 1)).astype(np.float32)

    order = np.argsort(dst, kind="stable")
    ssrc = src[order]
    sdst = dst[order]

    # store-range boundaries for every (core, window)
    store_lo = np.array([c * SHARD + w * WIN for c in range(NCORES) for w in range(WPC)]
                        + [N], dtype=np.int64)
    starts = np.searchsorted(sdst, store_lo)

    # dedup (src, window): one gathered row per distinct src in a window;
    # its A row carries every dst column (values summed for parallel edges)
    uniq = {}
    ucnt = np.zeros((NCORES, WPC), dtype=np.int64)
    for c in range(NCORES):
        for w in range(WPC):
            k = c * WPC + w
            sl = slice(starts[k], starts[k + 1])
            s_u, r_of = np.unique(ssrc[sl], return_inverse=True)
            uniq[(c, w)] = (s_u, r_of)
            ucnt[c, w] = len(s_u)

    T = np.maximum(1, -(-ucnt.max(axis=0) // 128))         # tiles per window
    offs = np.concatenate([[0], np.cumsum(T)])             # tile offsets
    total_T = int(offs[-1])

    ncalls = sum(len(_chunks(int(t))) for t in T)
    idx_blobs, A_blobs, cnt_blobs = [], [], []
    for c in range(NCORES):
        if STATIC_CNT:
            # valid, distinct padding indices (A columns are zero for them);
            # distinct rows avoid HBM hot-row serialization on padded gathers
            idxb = (np.arange(total_T * 128, dtype=np.int64).reshape(
                16, -1, order="F") % N).astype(np.int16)
        else:
            idxb = np.full((16, total_T * 8), -1, dtype=np.int16)
        Ab = np.zeros((128, total_T * 128), dtype=np.float32)
        cnts = np.zeros(ncalls, dtype=np.int32)
        ci = 0
        for w in range(WPC):
            k = c * WPC + w
            sl = slice(starts[k], starts[k + 1])
            d = sdst[sl]
            s_u, r_of = uniq[(c, w)]
            n = len(s_u)
            base = c * SHARD + (w * WIN if w < WPC - 1 else LAST_BASE)
            col = d - base
            np.add.at(Ab, ((r_of & 127), (offs[w] + (r_of >> 7)) * 128 + col), inv[d])
            r = np.arange(n)
            idxb[r % 16, offs[w] * 8 + r // 16] = s_u.astype(np.int16)
            for (t0, t1) in _chunks(int(T[w])):
                cnt = min(n, t1 * 128) - t0 * 128
                if cnt <= 0:
                    # keep one valid dummy so the call is never empty
                    pos = t0 * 128
                    idxb[pos % 16, offs[w] * 8 + pos // 16] = 0
                    cnt = 1
                cnts[ci] = cnt
                ci += 1
        assert ci == ncalls
        idx_blobs.append(np.tile(idxb, (8, 1)))
        A_blobs.append(Ab.astype(ml_dtypes.bfloat16))
        cnt_blobs.append(np.tile(cnts[None, :], (128, 1)))

    return [int(x) for x in T], [int(x) for x in offs], idx_blobs, A_blobs, cnt_blobs


def _build(Ts, offs, use_b, use_g, use_be, single_core=False, compile=True,
           repeat=1, skip=frozenset(), full_repeat=1):
    """Build + compile the SPMD program. Returns nc.

    single_core=True replaces the AllGather with a local DRAM copy (for
    cost-model timing via TimelineSim, which is single-core only).
    repeat>1 wraps each layer in a device-side For_i loop (timing only)."""
    total_T = sum(Ts)
    nc = bacc.Bacc("TRN2", target_bir_lowering=False, debug=False,
                   num_devices=1 if single_core else NCORES,
                   num_swdge_queues=NSWQ,
                   dynamic_dma_scratch_size=DMASCR)

    ncalls = sum(len(_chunks(t)) for t in Ts)
    xbf = nc.dram_tensor("xbf", [N, D], MSG_DT, kind="ExternalInput").ap()
    cntd = nc.dram_tensor("cnt", [128, ncalls], i32, kind="ExternalInput").ap()
    xsh = nc.dram_tensor("xsh", [SHARD, D], f32, kind="ExternalInput").ap()
    xshb = nc.dram_tensor("xshb", [SHARD, D], bf16, kind="ExternalInput").ap()
    idxb = nc.dram_tensor("idxb", [128, total_T * 8], i16, kind="ExternalInput").ap()
    Ab = nc.dram_tensor("Ab", [128, total_T * 128], bf16, kind="ExternalInput").ap()
    ident = nc.dram_tensor("ident", [128, 128], bf16, kind="ExternalInput").ap()
    # weights: [128, 2, 256] bf16 (rhs of the linear matmuls)
    wls = [nc.dram_tensor(f"w{i}l", [128, 2, D], bf16, kind="ExternalInput").ap()
           for i in (1, 2)]
    wrs = [nc.dram_tensor(f"w{i}r", [128, 2, D], bf16, kind="ExternalInput").ap()
           for i in (1, 2)]
    bias_in = {}
    for i in (1, 2):
        if use_b:
            bias_in[f"b{i}"] = nc.dram_tensor(f"b{i}", [128, D], f32, kind="ExternalInput").ap()
        if use_g:
            bias_in[f"g{i}"] = nc.dram_tensor(f"g{i}", [128, D], f32, kind="ExternalInput").ap()
        if use_be:
            bias_in[f"be{i}"] = nc.dram_tensor(f"be{i}", [128, D], f32, kind="ExternalInput").ap()
    out = nc.dram_tensor("out", [SHARD, D], f32, kind="ExternalOutput").ap()

    MAGIC = 0x5F3759DF

    with tile.TileContext(nc) as tc:
        with (
            tc.tile_pool(name="cst", bufs=1) as cst,
            tc.tile_pool(name="io", bufs=IOBUFS) as io,
            tc.tile_pool(name="mid", bufs=3) as mid,
            tc.tile_pool(name="sm", bufs=4) as sm,
            tc.tile_pool(name="ps", bufs=2, space="PSUM") as ps,
            tc.tile_pool(name="dram", bufs=1, space="DRAM") as dram,
        ):
            idx_sb = cst.tile([128, total_T * 8], i16)
            nc.sync.dma_start(idx_sb[:], idxb)
            cnt_sb = cst.tile([128, ncalls], i32)
            nc.sync.dma_start(cnt_sb[:], cntd)
            id_sb = cst.tile([128, 128], bf16)
            nc.sync.dma_start(id_sb[:], ident)
            wl_sb, wr_sb, bias_sb = [], [], {}
            for i in (0, 1):
                wl = cst.tile([128, 2, D], bf16, name=f"wl{i}")
                nc.sync.dma_start(wl[:], wls[i])
                wl_sb.append(wl)
                wr = cst.tile([128, 2, D], bf16, name=f"wr{i}")
                nc.sync.dma_start(wr[:], wrs[i])
                wr_sb.append(wr)
                for key, use in ((f"b{i+1}", use_b), (f"g{i+1}", use_g), (f"be{i+1}", use_be)):
                    if use:
                        bt = cst.tile([128, D], f32, name=f"bias_{key}")
                        nc.sync.dma_start(bt[:], bias_in[key])
                        bias_sb[key] = bt

            # gather skips trailing-negative-padded rows, so matmuls read
            # stale SBUF there (times all-zero A columns). Zero every msg pool
            # slot once at full extent so stale bytes can never be NaN/Inf.
            for j in range(IOBUFS):
                mz = io.tile([128, max(Ts), D], MSG_DT, tag="msg", name=f"msgz_{j}")
                nc.vector.memset(mz[:], 0.0)

            x1res = cst.tile([128, WPC, D], f32)
            x1f = dram.tile([WIN, D], f32)   # rows LAST_BASE..SHARD only
            agin = dram.tile([SHARD, D], MSG_DT)
            agout = dram.tile([N, D], MSG_DT)

            def emit_layer(layer, gsrc):
                ci = [0]
                for w in range(WPC):
                    T = Ts[w]
                    off = offs[w]
                    last = w == WPC - 1
                    base = (w * WIN) if not last else LAST_BASE
                    st_lo = w * WIN
                    n_st = WIN if not last else LAST_STORE
                    pofs = 0 if not last else WIN - LAST_STORE

                    msg = io.tile([128, T, D], MSG_DT, tag="msg", name=f"msg_{layer}_{w}")
                    if "gather" in skip:
                        nc.vector.memset(msg[:, 0, 0:8], 0.0)
                    if "gather" not in skip:
                        # Split tiles into NSWQ near-equal chunks (one per SWDGE
                        # queue; Q7 emission parallelizes across queues). Each
                        # chunk stays <= 8 tiles (1024 idxs/call limit). Exact
                        # per-core valid counts come from the cnt blob.
                        for qi, (t0, t1) in enumerate(_chunks(T)):
                            if STATIC_CNT:
                                cv = (t1 - t0) * 128
                            else:
                                cv = nc.gpsimd.value_load(cnt_sb[0:1, ci[0]:ci[0] + 1])
                            qn = ((w % 2) * 2 + qi) % NSWQ if QALT else qi % NSWQ
                            nc.gpsimd.dma_gather(msg[:, t0:t1, :], gsrc,
                                                 idx_sb[:, (off + t0) * 8:(off + t1) * 8],
                                                 (t1 - t0) * 128, cv, D,
                                                 single_packet=bool(SP),
                                                 queue_num=qn)
                            ci[0] += 1
                    A_sb = io.tile([128, T * 128], bf16, tag="A", name=f"A_{layer}_{w}")
                    if "aload" in skip:
                        nc.vector.memset(A_sb[:, 0:8], 0.0)
                    if "aload" not in skip:
                        nc.sync.dma_start(A_sb[:], Ab[:, off * 128:(off + T) * 128])
                    # x rows: f32 for the residual add, bf16 for the PE root
                    # transpose (bf16 transpose+matmul runs at 4x the f32 rate)
                    if layer == 0:
                        x_sb = io.tile([128, D], f32, tag="x", name=f"x_{layer}_{w}")
                        nc.sync.dma_start(x_sb[:], xsh[base:base + WIN, :])
                        xsl = lambda a, b: x_sb[:, a:b]
                        xb = io.tile([128, D], bf16, tag="xb", name=f"xb_{layer}_{w}")
                        nc.sync.dma_start(xb[:], xshb[base:base + WIN, :])
                    else:
                        if not last:
                            xsl = lambda a, b: x1res[:, w, a:b]
                        else:
                            x_sb = io.tile([128, D], f32, tag="x", name=f"x_{layer}_{w}")
                            nc.sync.dma_start(x_sb[:], x1f[:])
                            xsl = lambda a, b: x_sb[:, a:b]
                        xb = io.tile([128, D], bf16, tag="xb", name=f"xb_{layer}_{w}")
                        nc.scalar.activation(xb[:], xsl(0, D),
                                             mybir.ActivationFunctionType.Copy)

                    # aggregation: meanT halves accumulate over edge tiles
                    aggps = ps.tile([128, 2, 128], f32, tag="agg", name=f"agg_{layer}_{w}")
                    Teff = 1 if "agg" in skip else T
                    for hh in range(2):
                        for t in range(Teff):
                            nc.tensor.matmul(aggps[:, hh, :],
                                             msg[:, t, hh * 128:(hh + 1) * 128],
                                             A_sb[:, t * 128:(t + 1) * 128],
                                             start=(t == 0), stop=(t == Teff - 1))
                    aggsb = mid.tile([128, 2, 128], bf16, tag="aggsb", name=f"aggsb_{layer}_{w}")
                    nc.scalar.activation(aggsb[:], aggps[:],
                                         mybir.ActivationFunctionType.Copy)

                    # root transpose (all bf16)
                    if "roottr" in skip:
                        xtsb_t = aggsb
                    else:
                        xtps = ps.tile([128, 2, 128], bf16, tag="xt", name=f"xt_{layer}_{w}")
                        for hh in range(2):
                            nc.tensor.transpose(xtps[:, hh, :],
                                                xb[:, hh * 128:(hh + 1) * 128], id_sb[:])
                        xtsb = mid.tile([128, 2, 128], bf16, tag="xtsb", name=f"xtsb_{layer}_{w}")
                        nc.scalar.activation(xtsb[:], xtps[:],
                                             mybir.ActivationFunctionType.Copy)
                        xtsb_t = xtsb

                    # linear: h = meanT.T @ WlT + xT.T @ WrT (bf16 inputs, f32 PSUM)
                    hps = ps.tile([128, D], f32, tag="hP", name=f"hP_{layer}_{w}")
                    if "wr" in skip:
                        nc.tensor.matmul(hps[:], aggsb[:, 0, :], wl_sb[layer][:, 0, :],
                                         start=True, stop=False)
                        nc.tensor.matmul(hps[:], aggsb[:, 1, :], wl_sb[layer][:, 1, :],
                                         start=False, stop=True)
                    else:
                        nc.tensor.matmul(hps[:], aggsb[:, 0, :], wl_sb[layer][:, 0, :],
                                         start=True, stop=False)
                        nc.tensor.matmul(hps[:], aggsb[:, 1, :], wl_sb[layer][:, 1, :],
                                         start=False, stop=False)
                        wr_t = wl_sb[layer] if "roottr" in skip else wr_sb[layer]
                        nc.tensor.matmul(hps[:], xtsb_t[:, 0, :], wr_t[:, 0, :],
                                         start=False, stop=False)
                        nc.tensor.matmul(hps[:], xtsb_t[:, 1, :], wr_t[:, 1, :],
                                         start=False, stop=True)

                    if use_b:
                        h_sb = mid.tile([128, D], f32, tag="h", name=f"h_{layer}_{w}")
                        nc.vector.tensor_tensor(h_sb[:], hps[:],
                                                bias_sb[f"b{layer+1}"][:], Alu.add)
                        h_val = h_sb
                    else:
                        h_val = hps

                    # LayerNorm stats
                    if "ln" in skip:
                        y = mid.tile([128, D], f32, tag="y", name=f"y_{layer}_{w}")
                        nc.vector.tensor_copy(y[:], h_val[:])
                    else:
                        st6 = sm.tile([128, 6], f32, tag="st6", name=f"st6_{layer}_{w}")
                        nc.vector.bn_stats(st6[:], h_val[:])
                        mv = sm.tile([128, 2], f32, tag="mv", name=f"mv_{layer}_{w}")
                        nc.vector.bn_aggr(mv[:], st6[:])

                        # inv_std = rsqrt(var + eps): magic + Newton steps (DVE only)
                        va = sm.tile([128, 1], f32, tag="va", name=f"va_{layer}_{w}")
                        nc.vector.tensor_scalar(va[:], mv[:, 1:2], LN_EPS, None, Alu.add)
                        xi = sm.tile([128, 1], i32, tag="xi", name=f"xi_{layer}_{w}")
                        nc.vector.tensor_scalar(xi[:], va[:].bitcast(i32), 1, None,
                                                Alu.arith_shift_right)
                        nc.vector.tensor_scalar(xi[:], xi[:], MAGIC, -1,
                                                Alu.subtract, Alu.mult)
                        rs = sm.tile([128, 1], f32, tag="rs", name=f"rs_{layer}_{w}")
                        nc.vector.tensor_copy(rs[:], xi[:].bitcast(f32))
                        tmp = sm.tile([128, 1], f32, tag="tmp", name=f"tmp_{layer}_{w}")
                        n_newton = 1 if "newton1" in skip else NEWTON
                        for _ in range(n_newton):
                            nc.vector.tensor_tensor(tmp[:], rs[:], rs[:], Alu.mult)
                            nc.vector.tensor_tensor(tmp[:], tmp[:], va[:], Alu.mult)
                            nc.vector.tensor_scalar(tmp[:], tmp[:], -0.5, 1.5,
                                                    Alu.mult, Alu.add)
                            nc.vector.tensor_tensor(rs[:], rs[:], tmp[:], Alu.mult)

                    fuse_ln = ("ln" not in skip and not use_g and not use_be
                               and GELU_MODE == "gelu" and "gelu" not in skip)
                    if not fuse_ln and "ln" not in skip:
                        y = mid.tile([128, D], f32, tag="y", name=f"y_{layer}_{w}")
                        nc.vector.tensor_scalar(y[:], h_val[:], mv[:, 0:1], rs[:],
                                                Alu.subtract, Alu.mult)
                        if use_g:
                            nc.vector.tensor_tensor(y[:], y[:],
                                                    bias_sb[f"g{layer+1}"][:], Alu.mult)
                        if use_be:
                            nc.vector.tensor_tensor(y[:], y[:],
                                                    bias_sb[f"be{layer+1}"][:], Alu.add)

                    if layer == 0:
                        xn_ap = x1res[:, w, :]
                    else:
                        xn = mid.tile([128, D], f32, tag="xn", name=f"xn_{layer}_{w}")
                        xn_ap = xn[:]
                    if fuse_ln:
                        # gelu((h - mu) * rs) in one ACT op: scale=rs, bias=-mu*rs
                        nmurs = sm.tile([128, 1], f32, tag="nmurs", name=f"nm_{layer}_{w}")
                        nc.vector.tensor_scalar(nmurs[:], mv[:, 0:1], rs[:], -1.0,
                                                Alu.mult, Alu.mult)
                        gl = mid.tile([128, D], f32, tag="gl", name=f"gl_{layer}_{w}")
                        nc.scalar.activation(gl[:], h_val[:],
                                             mybir.ActivationFunctionType.Gelu,
                                             bias=nmurs[:, 0:1], scale=rs[:, 0:1])
                        nc.vector.tensor_tensor(xn_ap, xsl(0, D), gl[:], Alu.add)
                    elif "gelu" in skip:
                        nc.vector.tensor_tensor(xn_ap, xsl(0, D), y[:], Alu.add)
                    elif GELU_MODE in ("gelu", "tanh"):
                        fn = (mybir.ActivationFunctionType.Gelu if GELU_MODE == "gelu"
                              else mybir.ActivationFunctionType.Tanh)
                        gl = mid.tile([128, D], f32, tag="gl", name=f"gl_{layer}_{w}")
                        nc.scalar.activation(gl[:], y[:], fn)
                        nc.vector.tensor_tensor(xn_ap, xsl(0, D), gl[:], Alu.add)
                    else:
                        er = mid.tile([128, D], f32, tag="gl", name=f"gl_{layer}_{w}")
                        nc.scalar.activation(er[:], y[:],
                                             mybir.ActivationFunctionType.Erf,
                                             scale=float(1.0 / np.sqrt(2.0)))
                        # z = (er + 1) * y ; xn = 0.5*z + x
                        nc.vector.scalar_tensor_tensor(er[:], er[:], 1.0, y[:],
                                                       Alu.add, Alu.mult)
                        nc.vector.scalar_tensor_tensor(xn_ap, er[:], 0.5, xsl(0, D),
                                                       Alu.mult, Alu.add)

                    if "store" in skip:
                        pass
                    elif layer == 0:
                        if AGSYNC:
                            mc = mid.tile([128, D], MSG_DT, tag="mc", name=f"mc_{w}")
                            nc.scalar.activation(mc[:], x1res[:, w, :],
                                                 mybir.ActivationFunctionType.Copy)
                            nc.sync.dma_start(agin[st_lo:st_lo + n_st, :],
                                              mc[pofs:, :])
                        else:
                            nc.gpsimd.dma_start(agin[st_lo:st_lo + n_st, :],
                                                x1res[pofs:, w, :])
                        if w == WPC - 2:
                            # rows LAST_BASE..(WPC-1)*WIN live in slot w's tail
                            nc.sync.dma_start(
                                x1f[0:WIN - LAST_STORE, :],
                                x1res[WIN - (WIN - LAST_STORE):, w, :])
                        elif last:
                            nc.sync.dma_start(x1f[WIN - LAST_STORE:, :],
                                              x1res[pofs:, w, :])
                    else:
                        nc.sync.dma_start(out[st_lo:st_lo + n_st, :], xn[pofs:, :])

            for rep in range(full_repeat):
                for layer in (0, 1):
                    gsrc = xbf if layer == 0 else agout[:]
                    if repeat > 1:
                        with tc.For_i(0, repeat, 1):
                            emit_layer(layer, gsrc)
                    else:
                        emit_layer(layer, gsrc)

                    if layer == 0:
                        if single_core:
                            nc.sync.dma_start(agout[0:SHARD, :], agin[:])
                        else:
                            nc.gpsimd.collective_compute(
                                "AllGather", Alu.bypass,
                                replica_groups=[list(range(NCORES))],
                                ins=[agin.opt()], outs=[agout.opt()],
                            )

    if compile:
        nc.compile()
    return nc


def _prepare(inputs):
    edge_index = np.asarray(inputs["edge_index"])
    key = hash(edge_index.tobytes())
    if key in _cache:
        return _cache[key]

    Ts, offs, idx_blobs, A_blobs, cnt_blobs = _preprocess(edge_index)

    b1 = np.asarray(inputs["b1l"], dtype=np.float32)
    b2 = np.asarray(inputs["b2l"], dtype=np.float32)
    g1 = np.asarray(inputs["g1"], dtype=np.float32)
    g2 = np.asarray(inputs["g2"], dtype=np.float32)
    be1 = np.asarray(inputs["be1"], dtype=np.float32)
    be2 = np.asarray(inputs["be2"], dtype=np.float32)
    use_b = not (np.all(b1 == 0) and np.all(b2 == 0))
    use_g = not (np.all(g1 == 1) and np.all(g2 == 1))
    use_be = not (np.all(be1 == 0) and np.all(be2 == 0))

    nc = _build(Ts, offs, use_b, use_g, use_be)

    x = np.asarray(inputs["x"], dtype=np.float32)
    xbf = x.astype(MSG_NPDT)
    ident = np.eye(128, dtype=ml_dtypes.bfloat16)

    def wdev(W, dtype):
        # rhs[k, j] = W[j, k]; layout [128 part=k%?, 2 khalf, 256 j]
        WT = np.ascontiguousarray(np.asarray(W, dtype=np.float32).T)  # [k, j]
        return WT.reshape(2, 128, D).transpose(1, 0, 2).astype(dtype).copy()

    common = {
        "xbf": xbf,
        "ident": ident,
        "w1l": wdev(inputs["W1l"], ml_dtypes.bfloat16),
        "w2l": wdev(inputs["W2l"], ml_dtypes.bfloat16),
        "w1r": wdev(inputs["W1r"], ml_dtypes.bfloat16),
        "w2r": wdev(inputs["W2r"], ml_dtypes.bfloat16),
    }
    if use_b:
        common["b1"] = np.tile(b1[None, :], (128, 1))
        common["b2"] = np.tile(b2[None, :], (128, 1))
    if use_g:
        common["g1"] = np.tile(g1[None, :], (128, 1))
        common["g2"] = np.tile(g2[None, :], (128, 1))
    if use_be:
        common["be1"] = np.tile(be1[None, :], (128, 1))
        common["be2"] = np.tile(be2[None, :], (128, 1))

    in_maps = []
    for c in range(NCORES):
        m = dict(common)
        m["xsh"] = np.ascontiguousarray(x[c * SHARD:(c + 1) * SHARD, :])
        m["xshb"] = np.ascontiguousarray(
            x[c * SHARD:(c + 1) * SHARD, :].astype(ml_dtypes.bfloat16))
        m["idxb"] = idx_blobs[c]
        m["Ab"] = A_blobs[c]
        m["cnt"] = cnt_blobs[c]
        in_maps.append(m)

    _cache[key] = (nc, in_maps)
    return nc, in_maps


def _assemble(res):
    return np.concatenate([np.asarray(res.results[c]["out"], dtype=np.float32)
                           for c in range(NCORES)], axis=0)


def kernel(**inputs):
    nc, in_maps = _prepare(inputs)
    res = bass_utils.run_bass_kernel_spmd(nc, in_maps, core_ids=list(range(NCORES)))
    return _assemble(res)


def run_traced(**inputs):
    """Returns (output, exec_time_ns or None). For test harness use."""
    nc, in_maps = _prepare(inputs)
    try:
        res = bass_utils.run_bass_kernel_spmd(
            nc, in_maps, core_ids=list(range(NCORES)), trace=True)
        return _assemble(res), res.exec_time_ns
    except Exception as e:  # trace/profile infra can fail independently of the run
        print(f"traced run failed ({e}); falling back to untraced")
        res = bass_utils.run_bass_kernel_spmd(nc, in_maps, core_ids=list(range(NCORES)))
        return _assemble(res), None

